# revision 1
# baseline (speedup 1.0000x reference)
"""Grouped-Query Attention (B=2, T=2048, H=2048, 16 q-heads, 4 kv-heads, d=128,
causal) on 8 Trainium2 NeuronCores.

Sharding: core c = (batch b, kv-group g) with b = c // 4, g = c % 4.
Each core handles one batch element, one kv head, and its 4 q heads:
  - Q/K/V projections for its slice (tensor-parallel over heads)
  - causal attention for 4 q heads against the shared K/V head
  - partial o_proj (row-parallel): out_partial = O_heads @ w_o[:, cols].T
Host sums the 4 per-batch partials (the row-parallel all-reduce) and stacks.

Device layouts (chosen so no transposes are ever needed on-chip):
  QT, KT: [d=128, T]  (projection computed directly transposed)
  V:      [T-tile=128, d]
  scores: computed directly transposed as ST [k, q] via lhsT=KT_j, rhs=QT
  P = exp(ST/sqrt(d)) stays [k, q] and feeds PV as rhs -> OT [d, q] which is
  exactly the lhsT the o_proj needs. Row sums of P (softmax denominator) are
  computed broadcast via an all-ones [128,128] stationary matmul.
All matmul inputs bf16, PSUM accumulation fp32, softmax in fp32.
"""

import numpy as np
import ml_dtypes
from contextlib import ExitStack

import concourse.bass as bass
import concourse.mybir as mybir
import concourse.tile as tile
from concourse.bass_utils import run_bass_kernel_spmd

# ---------------------------------------------------------------------------
# Workaround for this compiler build's per-instruction sync-wait-slot limit
# (walrus setupSyncWait rejects >2 waits on an instruction). Post-process the
# serialized BIR: any instruction carrying more than 2 sem waits gets the
# excess moved onto injected same-engine Drain instructions placed directly
# before it (same queue, program order => identical semantics).
import json as _json

_WAIT_LIMITS = {}
_WAIT_LIMIT_DEFAULT = 1
_orig_to_json_bytes = bass.Bass.to_json_bytes


def _split_waits_json(bj: bytes) -> bytes:
    m = _json.loads(bj)
    ctr = 0
    changed = False
    for f in m["functions"]:
        for blk in f["blocks"]:
            out = []
            for inst in blk["instructions"]:
                si = inst.get("sync_info") or {}
                w = si.get("on_wait") or []
                lim = _WAIT_LIMITS.get(inst.get("opcode"), _WAIT_LIMIT_DEFAULT)
                if len(w) > lim:
                    changed = True
                    extra, keep = w[:-lim], w[-lim:]
                    si["on_wait"] = keep
                    for i in range(0, len(extra), 1):
                        ctr += 1
                        out.append({
                            "debug": inst.get("debug", 0),
                            "engine": inst["engine"],
                            "ins": [],
                            "is_reset_sema": False,
                            "name": f"I-wsplit-{ctr}",
                            "opcode": "Drain",
                            "outs": [],
                            "sync_info": {
                                "on_update": [],
                                "on_wait": extra[i:i + 1],
                            },
                        })
                out.append(inst)
            if changed:
                blk["instructions"] = out
    if not changed:
        return bj
    return _json.dumps(m).encode()


def _to_json_bytes_patched(self, *a, **k):
    return _split_waits_json(_orig_to_json_bytes(self, *a, **k))


bass.Bass.to_json_bytes = _to_json_bytes_patched
# ---------------------------------------------------------------------------

HIDDEN = 2048
N_HEADS = 16
N_KV = 4
HD = 128
B, T = 2, 2048
G = N_HEADS // N_KV          # q heads per core = 4
HC = HIDDEN // 128           # contraction chunks = 16
NCORES = 8
SCALE = HD ** -0.5

BF16 = mybir.dt.bfloat16
F32 = mybir.dt.float32

_CACHE = {}
LAST_RESULTS = None


def _build_program():
    nc = bass.Bass("TRN2")
    xT = nc.dram_tensor("xT", [HIDDEN, T], BF16, kind="ExternalInput")
    wq = nc.dram_tensor("wq", [HIDDEN, G * HD], BF16, kind="ExternalInput")
    wk = nc.dram_tensor("wk", [HIDDEN, HD], BF16, kind="ExternalInput")
    wv = nc.dram_tensor("wv", [HIDDEN, HD], BF16, kind="ExternalInput")
    wo = nc.dram_tensor("wo", [G * HD, HIDDEN], BF16, kind="ExternalInput")
    msk = nc.dram_tensor("msk", [128, G, 512], BF16, kind="ExternalInput")
    out = nc.dram_tensor("out", [T, HIDDEN], BF16, kind="ExternalOutput")

    xTv = xT.rearrange("(c p) t -> p c t", p=128)
    wqv = wq.rearrange("(c p) m -> p c m", p=128)
    wkv = wk.rearrange("(c p) d -> p c d", p=128)
    wvv = wv.rearrange("(c p) d -> p c d", p=128)
    wov = wo.rearrange("(h p) e -> p h e", p=128)

    EXP = mybir.ActivationFunctionType.Exp

    with tile.TileContext(nc) as tc, ExitStack() as ctx:
        sing = ctx.enter_context(tc.tile_pool(name="sing", bufs=1))
        ptp = ctx.enter_context(tc.tile_pool(name="ptp", bufs=4))
        vecp = ctx.enter_context(tc.tile_pool(name="vecp", bufs=2))
        otnp = ctx.enter_context(tc.tile_pool(name="otnp", bufs=8))
        outp = ctx.enter_context(tc.tile_pool(name="outp", bufs=3))
        psum = ctx.enter_context(tc.tile_pool(name="psum", bufs=2, space="PSUM"))

        xT_sb = sing.tile([128, HC, T], BF16)
        wq_sb = sing.tile([128, HC, G * HD], BF16)
        wk_sb = sing.tile([128, HC, HD], BF16)
        wv_sb = sing.tile([128, HC, HD], BF16)
        wo_sb = sing.tile([128, G, HIDDEN], BF16)
        msk_sb = sing.tile([128, G, 512], BF16)
        ones_sb = sing.tile([128, 128], BF16)
        qt_sb = sing.tile([128, G, T], BF16)
        kt_sb = sing.tile([128, T], BF16)
        v_sb = sing.tile([128, HC, HD], BF16)

        nc.vector.memset(ones_sb, 1.0)
        # DMA order: t4=0 working set (weights + first xT column block)
        # first so projections start ~16us in; the rest streams under
        # compute. wo is only needed ~130us in, so it goes last.
        for c in range(HC):
            nc.sync.dma_start(out=wq_sb[:, c, :], in_=wqv[:, c, :])
            nc.sync.dma_start(out=xT_sb[:, c, 0:512], in_=xTv[:, c, 0:512])
            nc.sync.dma_start(out=wk_sb[:, c, :], in_=wkv[:, c, :])
            nc.sync.dma_start(out=wv_sb[:, c, :], in_=wvv[:, c, :])
        nc.sync.dma_start(out=msk_sb, in_=msk[:, :, :])
        for t4 in range(1, 4):
            for c in range(HC):
                nc.sync.dma_start(out=xT_sb[:, c, t4 * 512:(t4 + 1) * 512],
                                  in_=xTv[:, c, t4 * 512:(t4 + 1) * 512])
        for h in range(G):
            nc.sync.dma_start(out=wo_sb[:, h, :], in_=wov[:, h, :])

        # ---- projections (per T-chunk, so attention can start early) ----
        for t4 in range(4):
            for h in range(G):
                qp = psum.tile([128, 512], F32, tag="st", bufs=2, name=f"qp_{h}_{t4}")
                for c in range(HC):
                    nc.tensor.matmul(
                        qp,
                        lhsT=wq_sb[:, c, h * HD:(h + 1) * HD],
                        rhs=xT_sb[:, c, t4 * 512:(t4 + 1) * 512],
                        start=(c == 0), stop=(c == HC - 1),
                    )
                nc.scalar.copy(qt_sb[:, h, t4 * 512:(t4 + 1) * 512], qp)
            kp = psum.tile([128, 512], F32, tag="st", bufs=2, name=f"kp_{t4}")
            for c in range(HC):
                nc.tensor.matmul(
                    kp,
                    lhsT=wk_sb[:, c, :],
                    rhs=xT_sb[:, c, t4 * 512:(t4 + 1) * 512],
                    start=(c == 0), stop=(c == HC - 1),
                )
            nc.scalar.copy(kt_sb[:, t4 * 512:(t4 + 1) * 512], kp)
            for tt in range(4 * t4, 4 * t4 + 4):
                vp = psum.tile([128, HD], F32, tag="st", bufs=2, name=f"vp_{tt}")
                for c in range(HC):
                    nc.tensor.matmul(
                        vp,
                        lhsT=xT_sb[:, c, tt * 128:(tt + 1) * 128],
                        rhs=wv_sb[:, c, :],
                        start=(c == 0), stop=(c == HC - 1),
                    )
                nc.scalar.copy(v_sb[:, tt, :], vp)

        # ---- attention + o_proj, per 512-wide q chunk ----
        for qc in range(4):
            qsl = slice(qc * 512, (qc + 1) * 512)
            njt = 4 * qc + 4
            otns = {}
            for h in range(G):
                ot = psum.tile([128, 512], F32, tag="ot", bufs=2, name=f"ot_{qc}_{h}")
                ls = psum.tile([128, 512], F32, tag="ls", bufs=2, name=f"ls_{qc}_{h}")
                for j in range(njt):
                    st = psum.tile([128, 512], F32, tag="st", bufs=2,
                                   name=f"st_{qc}_{h}_{j}")
                    nc.tensor.matmul(
                        st, lhsT=kt_sb[:, j * 128:(j + 1) * 128],
                        rhs=qt_sb[:, h, qsl], start=True, stop=True,
                    )
                    pt = ptp.tile([128, 512], BF16, tag="pt", bufs=16,
                                  name=f"pt_{qc}_{h}_{j}")
                    nc.scalar.activation(pt, st, EXP, scale=float(SCALE))
                    if j >= 4 * qc:
                        nc.vector.tensor_mul(pt, pt, msk_sb[:, j - 4 * qc, :])
                    nc.tensor.matmul(ot, lhsT=v_sb[:, j, :], rhs=pt,
                                     start=(j == 0), stop=(j == njt - 1))
                    nc.tensor.matmul(ls, lhsT=ones_sb, rhs=pt,
                                     start=(j == 0), stop=(j == njt - 1))
                lnl = vecp.tile([128, 512], F32, tag="lnl", bufs=2,
                                name=f"lnl_{qc}_{h}")
                nc.scalar.activation(lnl, ls, mybir.ActivationFunctionType.Ln)
                ots = vecp.tile([128, 512], F32, tag="ots", bufs=2,
                                name=f"ots_{qc}_{h}")
                nc.vector.tensor_copy(ots, ot)
                rec = vecp.tile([128, 512], F32, tag="rec", bufs=2,
                                name=f"rec_{qc}_{h}")
                nc.scalar.activation(rec, lnl, mybir.ActivationFunctionType.Exp,
                                     scale=-1.0)
                otn = otnp.tile([128, 512], BF16, tag="otn", bufs=8,
                                name=f"otn_{qc}_{h}")
                nc.vector.tensor_mul(otn, ots, rec)
                otns[h] = otn
            for tt in range(4):
                stage = outp.tile([128, HIDDEN], BF16, tag="stage", bufs=3,
                                  name=f"stage_{qc}_{tt}")
                for ec in range(4):
                    op = psum.tile([128, 512], F32, tag="op", bufs=2,
                                   name=f"op_{qc}_{tt}_{ec}")
                    for h in range(G):
                        nc.tensor.matmul(
                            op,
                            lhsT=otns[h][:, tt * 128:(tt + 1) * 128],
                            rhs=wo_sb[:, h, ec * 512:(ec + 1) * 512],
                            start=(h == 0), stop=(h == G - 1),
                        )
                    nc.vector.tensor_copy(stage[:, ec * 512:(ec + 1) * 512], op)
                r0 = qc * 512 + tt * 128
                nc.sync.dma_start(out=out[r0:r0 + 128, :], in_=stage)
    return nc


def _masks():
    kl = np.arange(128)[:, None, None]
    jj = np.arange(G)[None, :, None]
    ql = np.arange(512)[None, None, :]
    return (128 * jj + kl <= ql).astype(ml_dtypes.bfloat16)


def kernel(x, w_q, w_kv, w_o):
    global LAST_RESULTS
    if "nc" not in _CACHE:
        _CACHE["nc"] = _build_program()
        _CACHE["msk"] = _masks()
    nc = _CACHE["nc"]
    bf = ml_dtypes.bfloat16
    x = np.asarray(x, dtype=np.float32)
    w_q = np.asarray(w_q, dtype=np.float32)
    w_kv = np.asarray(w_kv, dtype=np.float32)
    w_o = np.asarray(w_o, dtype=np.float32)

    in_maps = []
    for c in range(NCORES):
        b, g = c // 4, c % 4
        in_maps.append({
            "xT": np.ascontiguousarray(x[b].T).astype(bf),
            "wq": np.ascontiguousarray(w_q[512 * g:512 * (g + 1), :].T).astype(bf),
            "wk": np.ascontiguousarray(w_kv[128 * g:128 * (g + 1), :].T).astype(bf),
            "wv": np.ascontiguousarray(
                w_kv[512 + 128 * g:512 + 128 * (g + 1), :].T).astype(bf),
            "wo": np.ascontiguousarray(w_o[:, 512 * g:512 * (g + 1)].T).astype(bf),
            "msk": _CACHE["msk"],
        })

    res = run_bass_kernel_spmd(nc, in_maps, core_ids=list(range(NCORES)))
    LAST_RESULTS = res
    outs = res.results
    o = [outs[c]["out"].astype(np.float32) for c in range(NCORES)]
    out = np.stack([o[0] + o[1] + o[2] + o[3], o[4] + o[5] + o[6] + o[7]])
    return out

